# revision 26
# baseline (speedup 1.0000x reference)
"""Self-contained TRN2 Bass kernel for the RGCN message-passing problem.

kernel(**inputs) takes the FULL unsharded inputs (text, src, dst, rel,
bases, comp, bias), shards edges by destination window across the 8
NeuronCores, runs the SPMD Bass program via run_bass_kernel_spmd, and
returns the full [64, 512, 256] float32 output.

Design (v2):
  - Edges are grouped by destination window (W=64 dst rows) and dealt
    to cores by descending window edge-count.  Since the edge indices
    are known on the host, the h[src] gather is done host-side: the
    device streams a pre-gathered [128, nchunks, 256] bf16 tensor with
    large sequential HWDGE DMAs (no SWDGE descriptor generation).
  - The per-edge scatter weights (comp[rel] placed at column
    b*W + dst%W) are built ON-CHIP from an 8-byte/edge metadata stream
    with two DVE ops per slab: onehot = is_equal(iota, dstloc) and
    w1h = onehot * compvals (broadcast APs).
  - Stage 1: per 128-edge chunk, 2 matmuls (G feature-half stationary,
    w1h moving) accumulate A[f, (w2,b,d)] for a PAIR of windows in one
    PSUM tile.
  - Stage 2: flipped — bases halves are the stationary operand (full
    128-wide PE), aggregated features stream as 256 dst columns per
    group of 4 windows.  Bias+ReLU fused in the scalar-engine
    activation; output written as [2, 128, dcore] (feature-major) and
    transposed back on the host.
"""

import numpy as np
import ml_dtypes

import concourse.bass as bass
import concourse.tile as tile
from concourse import bacc, mybir

F = 256       # in features
O = 256       # out features
NB = 3        # bases
W = 32        # dst rows per window
TSLAB = 16    # chunks per DMA slab (16 * 128 * 512B = 1 MiB)
GRP = 256 // W            # windows per stage-2 group (GRP*W = 256 dst cols)
PW = 384 // (NB * W)      # windows per stage-1 PSUM tile (384 f32 cols)


def build_program(n_nodes, slot_cws, n_cores=8):
    slot_cws = list(slot_cws)
    nw = len(slot_cws)
    assert nw % GRP == 0
    nchunks = sum(slot_cws)
    dcore = nw * W
    nslabs = -(-nchunks // TSLAB)
    npairs = nw // PW

    bf16 = mybir.dt.bfloat16
    f32 = mybir.dt.float32
    i16 = mybir.dt.int16

    # bf16 DRAM I/O breaks NEFF load under the PJRT path; all bf16 payloads
    # travel as int16 containers and are bitcast on-chip.
    nc = bacc.Bacc("TRN2", target_bir_lowering=False, debug=False,
                   num_devices=n_cores)
    gh_d = nc.dram_tensor("gh", [128, nchunks, F], i16,
                          kind="ExternalInput").ap()
    meta_d = nc.dram_tensor("meta", [128, nchunks, 4], i16,
                            kind="ExternalInput").ap()
    basesw_d = nc.dram_tensor("basesw", [128, NB, 2, 2, 128], i16,
                              kind="ExternalInput").ap()
    bias_d = nc.dram_tensor("bias", [128, 2], f32, kind="ExternalInput").ap()
    out_d = nc.dram_tensor("out", [2, 128, dcore], i16,
                           kind="ExternalOutput").ap()

    # chunk index -> (slot, is_first_chunk_of_slot, is_last_chunk_of_slot)
    chunk_slot = []
    for s, cw in enumerate(slot_cws):
        for c in range(cw):
            chunk_slot.append((s, c == 0, c == cw - 1))

    with tile.TileContext(nc) as tc:
        with (
            tc.tile_pool(name="const", bufs=1) as cpool,
            tc.tile_pool(name="gh", bufs=3) as gpool,
            tc.tile_pool(name="w1h", bufs=2) as wpool,
            tc.tile_pool(name="oh", bufs=2) as ohpool,
            tc.tile_pool(name="stg", bufs=2) as apool,
            tc.tile_pool(name="ost", bufs=4) as opool,
            tc.tile_pool(name="ps1", bufs=2, space="PSUM") as ps1,
            tc.tile_pool(name="ps2", bufs=2, space="PSUM") as ps2,
        ):
            # ---- prologue: constants ----
            # slab 0's metadata slice + G data go first so the PE can start
            # as early as possible; the rest of the constants follow.
            t0 = min(nchunks, TSLAB)
            meta_sb = cpool.tile([128, nchunks, 4], i16)
            nc.sync.dma_start(meta_sb[:, 0:t0, :], meta_d[:, 0:t0, :])
            meta_bf = meta_sb[:].bitcast(bf16)
            iota_sb = cpool.tile([128, W], bf16)
            nc.gpsimd.iota(iota_sb[:], pattern=[[1, W]], base=0,
                           channel_multiplier=0,
                           allow_small_or_imprecise_dtypes=True)

            slab_tiles = {}

            def ensure_slab(si):
                """Issue the G-slab DMA + on-chip w1h build for slab si."""
                if si in slab_tiles or si >= nslabs:
                    return
                lo = si * TSLAB
                hi = min(nchunks, lo + TSLAB)
                t = hi - lo
                G = gpool.tile([128, TSLAB, F], i16, tag="G", name="G")
                # alternate slabs across the two HWDGE rings (SP / ACT)
                ring = nc.sync if si % 2 == 0 else nc.scalar
                ring.dma_start(G[:, 0:t, :], gh_d[:, lo:hi, :])
                oh = ohpool.tile([128, TSLAB, W], bf16, tag="oh", name="oh")
                dstl = meta_bf[:, lo:hi, 0:1].broadcast_to([128, t, W])
                iota_b = iota_sb[:].unsqueeze(1).broadcast_to([128, t, W])
                nc.vector.tensor_tensor(oh[:, 0:t, :], iota_b, dstl,
                                        mybir.AluOpType.is_equal)
                w1h = wpool.tile([128, TSLAB, NB, W], bf16, tag="W", name="W")
                oh_b = oh[:, 0:t, :].unsqueeze(2).broadcast_to(
                    [128, t, NB, W])
                cv_b = meta_bf[:, lo:hi, 1:4].unsqueeze(3).broadcast_to(
                    [128, t, NB, W])
                nc.vector.tensor_tensor(w1h[:, 0:t, :, :], oh_b, cv_b,
                                        mybir.AluOpType.mult)
                slab_tiles[si] = (G, w1h)

            def emit_stage2_mm(g):
                """Matmul group g's staged features against the bases."""
                stg = staging[g]
                for ohalf in range(2):
                    p2 = ps2.tile([128, GRP * W], f32, tag=f"p2o{ohalf}",
                                  name=f"p2o{ohalf}")
                    k = 0
                    for b in range(NB):
                        for h in range(2):
                            nc.tensor.matmul(
                                p2[:],
                                basesw_sb[:, b, h, ohalf, :].bitcast(bf16),
                                stg[:, h, b, :, :],
                                start=(k == 0), stop=(k == 2 * NB - 1))
                            k += 1
                    p2_of_group[(g, ohalf)] = p2
                del staging[g]

            def emit_relu_out(g):
                """ReLU+bias and output DMA for group g (deferred so the
                scalar queue's casts never sit behind a relu)."""
                for ohalf in range(2):
                    p2 = p2_of_group.pop((g, ohalf))
                    osb = opool.tile([128, GRP * W], bf16, tag=f"osb{ohalf}",
                                     name=f"osb{ohalf}")
                    nc.scalar.activation(
                        osb[:], p2[:], mybir.ActivationFunctionType.Relu,
                        bias=bias_sb[:, ohalf:ohalf + 1])
                    # idle GpSimd SWDGE queue: stay off the G-slab rings
                    nc.gpsimd.dma_start(
                        out_d[ohalf, :, g * GRP * W:(g + 1) * GRP * W],
                        osb[:].bitcast(i16))

            # ---- main pipeline over window pairs ----
            staging = {}
            p2_of_group = {}
            cglob = 0
            ensure_slab(0)
            basesw_sb = cpool.tile([128, NB, 2, 2, 128], i16)
            nc.gpsimd.dma_start(basesw_sb[:], basesw_d[:])
            bias_sb = cpool.tile([128, 2], f32)
            nc.gpsimd.dma_start(bias_sb[:], bias_d[:])
            if nchunks > t0:
                nc.gpsimd.dma_start(meta_sb[:, t0:, :], meta_d[:, t0:, :])
            ensure_slab(1)
            for j in range(npairs):
                p1 = [ps1.tile([128, PW, NB, W], f32, tag=f"p1h{h}",
                               name=f"p1h{h}") for h in range(2)]
                for w01 in range(PW):
                    s = PW * j + w01
                    for c in range(slot_cws[s]):
                        si, cloc = divmod(cglob, TSLAB)
                        ensure_slab(si)
                        ensure_slab(si + 1)
                        G, w1h = slab_tiles[si]
                        first = (c == 0)
                        last = (c == slot_cws[s] - 1)
                        for h in range(2):
                            nc.tensor.matmul(
                                p1[h][:, w01, :, :],
                                G[:, cloc, h * 128:(h + 1) * 128].bitcast(
                                    bf16),
                                w1h[:, cloc, :, :],
                                start=first, stop=last)
                        cglob += 1
                # defer stage2 of group (j-2)//2 to keep PE fed
                if j >= 2 and j % 2 == 0:
                    emit_stage2_mm((j - 2) // 2)
                if j >= 3 and j % 2 == 1:
                    emit_relu_out((j - 3) // 2)
                g = j // 2
                if g not in staging:
                    staging[g] = apool.tile([128, 2, NB, GRP, W], bf16,
                                            tag="stg", name="stg")
                for h in range(2):
                    a = PW * (j % 2)
                    dst_ap = staging[g][:, h, :, a:a + PW,
                                        :].transpose([0, 2, 1, 3])
                    # psum->staging casts ride the mostly-idle scalar engine
                    nc.scalar.copy(dst_ap, p1[h][:])
            for g in sorted(staging):
                emit_stage2_mm(g)
            for g in sorted(set(g for g, _ in p2_of_group)):
                emit_relu_out(g)

    nc.compile()
    return nc


def host_prep(h_bf, src, dst, rel, comp, n_nodes, n_cores):
    """Sort/deal/pad edges; pre-gather h and build the metadata stream."""
    dcore = n_nodes // n_cores
    nw = dcore // W
    ngw = n_cores * nw
    gw = (dst // W).astype(np.int64)
    order = np.argsort(gw, kind="stable")
    counts = np.bincount(gw, minlength=ngw)
    starts = np.concatenate([[0], np.cumsum(counts)])

    # deal windows to cores by descending count; slot capacity = group max
    ranked = np.argsort(-counts, kind="stable")
    slot_cws = [max(1, -(-int(counts[ranked[n_cores * i]]) // 128))
                for i in range(nw)]
    nchunks = sum(slot_cws)
    epad = nchunks * 128

    w_edge = comp[rel].astype(ml_dtypes.bfloat16)        # [E, NB]
    dstloc = (dst % W).astype(np.float32).astype(ml_dtypes.bfloat16)

    gh = np.zeros((n_cores, 128, nchunks, F), np.int16)
    meta = np.zeros((n_cores, 128, nchunks, 4), ml_dtypes.bfloat16)
    win_of_slot = np.zeros((n_cores, nw), np.int64)

    slot_base = np.concatenate([[0], np.cumsum(slot_cws)])[:-1]
    srcs_flat = np.zeros((n_cores, epad), np.int64)
    meta_flat = np.zeros((n_cores, epad, 4), ml_dtypes.bfloat16)
    valid = np.zeros((n_cores, epad), bool)
    for k in range(n_cores):
        for i in range(nw):
            wid = int(ranked[n_cores * i + k])
            win_of_slot[k, i] = wid
            es = order[starts[wid]:starts[wid + 1]]
            base = slot_base[i] * 128
            n = len(es)
            srcs_flat[k, base:base + n] = src[es]
            valid[k, base:base + n] = True
            meta_flat[k, base:base + n, 0] = dstloc[es]
            meta_flat[k, base:base + n, 1:4] = w_edge[es]
    for k in range(n_cores):
        g = h_bf[srcs_flat[k]]                           # [epad, F] int16
        g[~valid[k]] = 0
        gh[k] = g.reshape(nchunks, 128, F).transpose(1, 0, 2)
        meta[k] = meta_flat[k].reshape(nchunks, 128, 4).transpose(1, 0, 2)
    return gh, meta, tuple(slot_cws), win_of_slot


def rgcn_kernel(text, src, dst, rel, bases, comp, bias, n_cores=8,
                run_fn=None, nc_cache={}):
    """Full-input kernel: shard, run on 8 cores, reassemble output."""
    Bt, St, INF = text.shape
    n_nodes = Bt * St
    h = text.reshape(n_nodes, INF)

    src = np.asarray(src).astype(np.int64)
    dst = np.asarray(dst).astype(np.int64)
    rel = np.asarray(rel).astype(np.int64)
    bases_np = np.asarray(bases, np.float32)
    comp_np = np.asarray(comp, np.float32)
    bias_np = np.asarray(bias, np.float32)

    h_bf = np.asarray(h, np.float32).astype(ml_dtypes.bfloat16).view(np.int16)
    gh, meta, slot_cws, win_of_slot = host_prep(
        h_bf, src, dst, rel, comp_np, n_nodes, n_cores)
    key = (n_nodes, slot_cws, n_cores)
    if key not in nc_cache:
        nc_cache[key] = build_program(n_nodes, slot_cws, n_cores)
    nc = nc_cache[key]

    # bases[b, f, o] -> basesw[p, b, h, oh, q] with f = h*128+p, o = oh*128+q
    bw = bases_np.astype(ml_dtypes.bfloat16).view(np.int16)
    basesw = np.ascontiguousarray(
        bw.reshape(NB, 2, 128, 2, 128).transpose(2, 0, 1, 3, 4))
    bias_w = np.ascontiguousarray(
        bias_np.reshape(2, 128).T.astype(np.float32))

    in_maps = [
        dict(gh=gh[k], meta=meta[k].view(np.int16), basesw=basesw,
             bias=bias_w)
        for k in range(n_cores)
    ]
    from concourse.bass_utils import run_bass_kernel_spmd
    if run_fn is None:
        res = run_bass_kernel_spmd(nc, in_maps, list(range(n_cores)))
        outs = [res.results[k]["out"] for k in range(n_cores)]
    else:
        outs = run_fn(nc, in_maps)

    out = np.zeros((n_nodes, O), np.float32)
    nw = len(slot_cws)
    for k in range(n_cores):
        ok = outs[k].view(ml_dtypes.bfloat16).astype(np.float32)
        ok = ok.reshape(O, nw, W)                        # [o, slot, d]
        for i in range(nw):
            wid = win_of_slot[k][i]
            out[wid * W:(wid + 1) * W] = ok[:, i, :].T
    return out.reshape(Bt, St, O)


_NC_CACHE = {}


def kernel(text, src, dst, rel, bases, comp, bias):
    out = rgcn_kernel(
        np.asarray(text, np.float32),
        np.asarray(src), np.asarray(dst), np.asarray(rel),
        np.asarray(bases, np.float32), np.asarray(comp, np.float32),
        np.asarray(bias, np.float32),
        n_cores=8, nc_cache=_NC_CACHE)
    return np.ascontiguousarray(out, np.float32)


# revision 30
# speedup vs baseline: 1.1506x; 1.1506x over previous
"""Self-contained TRN2 Bass kernel for the RGCN message-passing problem.

kernel(**inputs) takes the FULL unsharded inputs (text, src, dst, rel,
bases, comp, bias), shards edges by destination window across the 8
NeuronCores, runs the SPMD Bass program via run_bass_kernel_spmd, and
returns the full [64, 512, 256] float32 output.

Design (v2):
  - Edges are grouped by destination window (W=64 dst rows) and dealt
    to cores by descending window edge-count.  Since the edge indices
    are known on the host, the h[src] gather is done host-side: the
    device streams a pre-gathered [128, nchunks, 256] bf16 tensor with
    large sequential HWDGE DMAs (no SWDGE descriptor generation).
  - The per-edge scatter weights (comp[rel] placed at column
    b*W + dst%W) are built ON-CHIP from an 8-byte/edge metadata stream
    with two DVE ops per slab: onehot = is_equal(iota, dstloc) and
    w1h = onehot * compvals (broadcast APs).
  - Stage 1: per 128-edge chunk, 2 matmuls (G feature-half stationary,
    w1h moving) accumulate A[f, (w2,b,d)] for a PAIR of windows in one
    PSUM tile.
  - Stage 2: flipped — bases halves are the stationary operand (full
    128-wide PE), aggregated features stream as 256 dst columns per
    group of 4 windows.  Bias+ReLU fused in the scalar-engine
    activation; output written as [2, 128, dcore] (feature-major) and
    transposed back on the host.
"""

import numpy as np
import ml_dtypes

import concourse.bass as bass
import concourse.tile as tile
from concourse import bacc, mybir

F = 256       # in features
O = 256       # out features
NB = 3        # bases
W = 32        # dst rows per window
TSLAB = 32    # chunks per DMA slab (32 * 128 * 512B = 2 MiB)
GRP = 256 // W            # windows per stage-2 group (GRP*W = 256 dst cols)
PW = 384 // (NB * W)      # windows per stage-1 PSUM tile (384 f32 cols)


def build_program(n_nodes, slot_cws, n_cores=8):
    slot_cws = list(slot_cws)
    nw = len(slot_cws)
    assert nw % GRP == 0
    nchunks = sum(slot_cws)
    dcore = nw * W
    nslabs = -(-nchunks // TSLAB)
    npairs = nw // PW

    bf16 = mybir.dt.bfloat16
    f32 = mybir.dt.float32
    i16 = mybir.dt.int16

    # bf16 DRAM I/O breaks NEFF load under the PJRT path; all bf16 payloads
    # travel as int16 containers and are bitcast on-chip.
    nc = bacc.Bacc("TRN2", target_bir_lowering=False, debug=False,
                   num_devices=n_cores)
    gh_d = nc.dram_tensor("gh", [128, nchunks, F], i16,
                          kind="ExternalInput").ap()
    meta_d = nc.dram_tensor("meta", [128, nchunks, 4], i16,
                            kind="ExternalInput").ap()
    basesw_d = nc.dram_tensor("basesw", [128, NB, 2, 2, 128], i16,
                              kind="ExternalInput").ap()
    bias_d = nc.dram_tensor("bias", [128, 2], f32, kind="ExternalInput").ap()
    out_d = nc.dram_tensor("out", [2, 128, dcore], i16,
                           kind="ExternalOutput").ap()

    # chunk index -> (slot, is_first_chunk_of_slot, is_last_chunk_of_slot)
    chunk_slot = []
    for s, cw in enumerate(slot_cws):
        for c in range(cw):
            chunk_slot.append((s, c == 0, c == cw - 1))

    with tile.TileContext(nc) as tc:
        with (
            tc.tile_pool(name="const", bufs=1) as cpool,
            tc.tile_pool(name="gh", bufs=3) as gpool,
            tc.tile_pool(name="w1h", bufs=2) as wpool,
            tc.tile_pool(name="oh", bufs=2) as ohpool,
            tc.tile_pool(name="stg", bufs=2) as apool,
            tc.tile_pool(name="ost", bufs=4) as opool,
            tc.tile_pool(name="ps1", bufs=2, space="PSUM") as ps1,
            tc.tile_pool(name="ps2", bufs=2, space="PSUM") as ps2,
        ):
            # ---- prologue: constants ----
            # slab 0's metadata slice + G data go first so the PE can start
            # as early as possible; the rest of the constants follow.
            t0 = min(nchunks, TSLAB)
            meta_sb = cpool.tile([128, nchunks, 4], i16)
            nc.sync.dma_start(meta_sb[:, 0:t0, :], meta_d[:, 0:t0, :])
            meta_bf = meta_sb[:].bitcast(bf16)
            iota_sb = cpool.tile([128, W], bf16)
            nc.gpsimd.iota(iota_sb[:], pattern=[[1, W]], base=0,
                           channel_multiplier=0,
                           allow_small_or_imprecise_dtypes=True)

            slab_tiles = {}

            def ensure_slab(si):
                """Issue the G-slab DMA + on-chip w1h build for slab si."""
                if si in slab_tiles or si >= nslabs:
                    return
                lo = si * TSLAB
                hi = min(nchunks, lo + TSLAB)
                t = hi - lo
                G = gpool.tile([128, TSLAB, F], i16, tag="G", name="G")
                mid = (t + 1) // 2
                nc.sync.dma_start(G[:, 0:mid, :], gh_d[:, lo:lo + mid, :])
                nc.sync.dma_start(G[:, mid:t, :], gh_d[:, lo + mid:hi, :])
                oh = ohpool.tile([128, TSLAB, W], bf16, tag="oh", name="oh")
                dstl = meta_bf[:, lo:hi, 0:1].broadcast_to([128, t, W])
                iota_b = iota_sb[:].unsqueeze(1).broadcast_to([128, t, W])
                nc.vector.tensor_tensor(oh[:, 0:t, :], iota_b, dstl,
                                        mybir.AluOpType.is_equal)
                w1h = wpool.tile([128, TSLAB, NB, W], bf16, tag="W", name="W")
                oh_b = oh[:, 0:t, :].unsqueeze(2).broadcast_to(
                    [128, t, NB, W])
                cv_b = meta_bf[:, lo:hi, 1:4].unsqueeze(3).broadcast_to(
                    [128, t, NB, W])
                nc.vector.tensor_tensor(w1h[:, 0:t, :, :], oh_b, cv_b,
                                        mybir.AluOpType.mult)
                slab_tiles[si] = (G, w1h)

            def emit_stage2_mm(g):
                """Matmul group g's staged features against the bases."""
                stg = staging[g]
                for ohalf in range(2):
                    p2 = ps2.tile([128, GRP * W], f32, tag=f"p2o{ohalf}",
                                  name=f"p2o{ohalf}")
                    k = 0
                    for b in range(NB):
                        for h in range(2):
                            nc.tensor.matmul(
                                p2[:],
                                basesw_sb[:, b, h, ohalf, :].bitcast(bf16),
                                stg[:, h, b, :, :],
                                start=(k == 0), stop=(k == 2 * NB - 1))
                            k += 1
                    p2_of_group[(g, ohalf)] = p2
                del staging[g]

            def emit_relu_out(g):
                """ReLU+bias and output DMA for group g (deferred so the
                scalar queue's casts never sit behind a relu)."""
                for ohalf in range(2):
                    p2 = p2_of_group.pop((g, ohalf))
                    osb = opool.tile([128, GRP * W], bf16, tag=f"osb{ohalf}",
                                     name=f"osb{ohalf}")
                    nc.scalar.activation(
                        osb[:], p2[:], mybir.ActivationFunctionType.Relu,
                        bias=bias_sb[:, ohalf:ohalf + 1])
                    # scalar-engine DGE queue: don't sit behind 2MB G slabs
                    nc.scalar.dma_start(
                        out_d[ohalf, :, g * GRP * W:(g + 1) * GRP * W],
                        osb[:].bitcast(i16))

            # ---- main pipeline over window pairs ----
            staging = {}
            p2_of_group = {}
            cglob = 0
            ensure_slab(0)
            basesw_sb = cpool.tile([128, NB, 2, 2, 128], i16)
            nc.sync.dma_start(basesw_sb[:], basesw_d[:])
            bias_sb = cpool.tile([128, 2], f32)
            nc.sync.dma_start(bias_sb[:], bias_d[:])
            if nchunks > t0:
                nc.sync.dma_start(meta_sb[:, t0:, :], meta_d[:, t0:, :])
            ensure_slab(1)
            for j in range(npairs):
                p1 = [ps1.tile([128, PW, NB, W], f32, tag=f"p1h{h}",
                               name=f"p1h{h}") for h in range(2)]
                for w01 in range(PW):
                    s = PW * j + w01
                    for c in range(slot_cws[s]):
                        si, cloc = divmod(cglob, TSLAB)
                        ensure_slab(si)
                        ensure_slab(si + 1)
                        G, w1h = slab_tiles[si]
                        first = (c == 0)
                        last = (c == slot_cws[s] - 1)
                        for h in range(2):
                            nc.tensor.matmul(
                                p1[h][:, w01, :, :],
                                G[:, cloc, h * 128:(h + 1) * 128].bitcast(
                                    bf16),
                                w1h[:, cloc, :, :],
                                start=first, stop=last)
                        cglob += 1
                # defer stage2 of group (j-2)//2 to keep PE fed
                if j >= 2 and j % 2 == 0:
                    emit_stage2_mm((j - 2) // 2)
                if j >= 3 and j % 2 == 1:
                    emit_relu_out((j - 3) // 2)
                g = j // 2
                if g not in staging:
                    staging[g] = apool.tile([128, 2, NB, GRP, W], bf16,
                                            tag="stg", name="stg")
                for h in range(2):
                    a = PW * (j % 2)
                    dst_ap = staging[g][:, h, :, a:a + PW,
                                        :].transpose([0, 2, 1, 3])
                    # psum->staging casts ride the mostly-idle scalar engine
                    nc.scalar.copy(dst_ap, p1[h][:])
            for g in sorted(staging):
                emit_stage2_mm(g)
            for g in sorted(set(g for g, _ in p2_of_group)):
                emit_relu_out(g)

    nc.compile()
    return nc


def host_prep(h_bf, src, dst, rel, comp, n_nodes, n_cores):
    """Sort/deal/pad edges; pre-gather h and build the metadata stream."""
    dcore = n_nodes // n_cores
    nw = dcore // W
    ngw = n_cores * nw
    gw = (dst // W).astype(np.int64)
    order = np.argsort(gw, kind="stable")
    counts = np.bincount(gw, minlength=ngw)
    starts = np.concatenate([[0], np.cumsum(counts)])

    # deal windows to cores by descending count; slot capacity = group max
    ranked = np.argsort(-counts, kind="stable")
    slot_cws = [max(1, -(-int(counts[ranked[n_cores * i]]) // 128))
                for i in range(nw)]
    nchunks = sum(slot_cws)
    epad = nchunks * 128

    w_edge = comp[rel].astype(ml_dtypes.bfloat16)        # [E, NB]
    dstloc = (dst % W).astype(np.float32).astype(ml_dtypes.bfloat16)

    gh = np.zeros((n_cores, 128, nchunks, F), np.int16)
    meta = np.zeros((n_cores, 128, nchunks, 4), ml_dtypes.bfloat16)
    win_of_slot = np.zeros((n_cores, nw), np.int64)

    slot_base = np.concatenate([[0], np.cumsum(slot_cws)])[:-1]
    srcs_flat = np.zeros((n_cores, epad), np.int64)
    meta_flat = np.zeros((n_cores, epad, 4), ml_dtypes.bfloat16)
    valid = np.zeros((n_cores, epad), bool)
    for k in range(n_cores):
        for i in range(nw):
            wid = int(ranked[n_cores * i + k])
            win_of_slot[k, i] = wid
            es = order[starts[wid]:starts[wid + 1]]
            base = slot_base[i] * 128
            n = len(es)
            srcs_flat[k, base:base + n] = src[es]
            valid[k, base:base + n] = True
            meta_flat[k, base:base + n, 0] = dstloc[es]
            meta_flat[k, base:base + n, 1:4] = w_edge[es]
    for k in range(n_cores):
        g = h_bf[srcs_flat[k]]                           # [epad, F] int16
        g[~valid[k]] = 0
        gh[k] = g.reshape(nchunks, 128, F).transpose(1, 0, 2)
        meta[k] = meta_flat[k].reshape(nchunks, 128, 4).transpose(1, 0, 2)
    return gh, meta, tuple(slot_cws), win_of_slot


def rgcn_kernel(text, src, dst, rel, bases, comp, bias, n_cores=8,
                run_fn=None, nc_cache={}):
    """Full-input kernel: shard, run on 8 cores, reassemble output."""
    Bt, St, INF = text.shape
    n_nodes = Bt * St
    h = text.reshape(n_nodes, INF)

    src = np.asarray(src).astype(np.int64)
    dst = np.asarray(dst).astype(np.int64)
    rel = np.asarray(rel).astype(np.int64)
    bases_np = np.asarray(bases, np.float32)
    comp_np = np.asarray(comp, np.float32)
    bias_np = np.asarray(bias, np.float32)

    h_bf = np.asarray(h, np.float32).astype(ml_dtypes.bfloat16).view(np.int16)
    gh, meta, slot_cws, win_of_slot = host_prep(
        h_bf, src, dst, rel, comp_np, n_nodes, n_cores)
    key = (n_nodes, slot_cws, n_cores)
    if key not in nc_cache:
        nc_cache[key] = build_program(n_nodes, slot_cws, n_cores)
    nc = nc_cache[key]

    # bases[b, f, o] -> basesw[p, b, h, oh, q] with f = h*128+p, o = oh*128+q
    bw = bases_np.astype(ml_dtypes.bfloat16).view(np.int16)
    basesw = np.ascontiguousarray(
        bw.reshape(NB, 2, 128, 2, 128).transpose(2, 0, 1, 3, 4))
    bias_w = np.ascontiguousarray(
        bias_np.reshape(2, 128).T.astype(np.float32))

    in_maps = [
        dict(gh=gh[k], meta=meta[k].view(np.int16), basesw=basesw,
             bias=bias_w)
        for k in range(n_cores)
    ]
    from concourse.bass_utils import run_bass_kernel_spmd
    if run_fn is None:
        res = run_bass_kernel_spmd(nc, in_maps, list(range(n_cores)))
        outs = [res.results[k]["out"] for k in range(n_cores)]
    else:
        outs = run_fn(nc, in_maps)

    out = np.zeros((n_nodes, O), np.float32)
    nw = len(slot_cws)
    for k in range(n_cores):
        ok = outs[k].view(ml_dtypes.bfloat16).astype(np.float32)
        ok = ok.reshape(O, nw, W)                        # [o, slot, d]
        for i in range(nw):
            wid = win_of_slot[k][i]
            out[wid * W:(wid + 1) * W] = ok[:, i, :].T
    return out.reshape(Bt, St, O)


_NC_CACHE = {}


def kernel(text, src, dst, rel, bases, comp, bias):
    out = rgcn_kernel(
        np.asarray(text, np.float32),
        np.asarray(src), np.asarray(dst), np.asarray(rel),
        np.asarray(bases, np.float32), np.asarray(comp, np.float32),
        np.asarray(bias, np.float32),
        n_cores=8, nc_cache=_NC_CACHE)
    return np.ascontiguousarray(out, np.float32)


# revision 34
# speedup vs baseline: 1.1828x; 1.0279x over previous
"""Self-contained TRN2 Bass kernel for the RGCN message-passing problem.

kernel(**inputs) takes the FULL unsharded inputs (text, src, dst, rel,
bases, comp, bias), shards edges by destination window across the 8
NeuronCores, runs the SPMD Bass program via run_bass_kernel_spmd, and
returns the full [64, 512, 256] float32 output.

Design (v2):
  - Edges are grouped by destination window (W=64 dst rows) and dealt
    to cores by descending window edge-count.  Since the edge indices
    are known on the host, the h[src] gather is done host-side: the
    device streams a pre-gathered [128, nchunks, 256] bf16 tensor with
    large sequential HWDGE DMAs (no SWDGE descriptor generation).
  - The per-edge scatter weights (comp[rel] placed at column
    b*W + dst%W) are built ON-CHIP from an 8-byte/edge metadata stream
    with two DVE ops per slab: onehot = is_equal(iota, dstloc) and
    w1h = onehot * compvals (broadcast APs).
  - Stage 1: per 128-edge chunk, 2 matmuls (G feature-half stationary,
    w1h moving) accumulate A[f, (w2,b,d)] for a PAIR of windows in one
    PSUM tile.
  - Stage 2: flipped — bases halves are the stationary operand (full
    128-wide PE), aggregated features stream as 256 dst columns per
    group of 4 windows.  Bias+ReLU fused in the scalar-engine
    activation; output written as [2, 128, dcore] (feature-major) and
    transposed back on the host.
"""

import numpy as np
import ml_dtypes

import concourse.bass as bass
import concourse.tile as tile
from concourse import bacc, mybir

F = 256       # in features
O = 256       # out features
NB = 3        # bases
W = 32        # dst rows per window
TSLAB = 32    # chunks per DMA slab (32 * 128 * 512B = 2 MiB)
GRP = 256 // W            # windows per stage-2 group (GRP*W = 256 dst cols)
PW = 384 // (NB * W)      # windows per stage-1 PSUM tile (384 f32 cols)


def build_program(n_nodes, slot_cws, n_cores=8):
    slot_cws = list(slot_cws)
    nw = len(slot_cws)
    assert nw % GRP == 0
    nchunks = sum(slot_cws)
    dcore = nw * W
    nslabs = -(-nchunks // TSLAB)
    npairs = nw // PW

    bf16 = mybir.dt.bfloat16
    f32 = mybir.dt.float32
    i16 = mybir.dt.int16

    # bf16 DRAM I/O breaks NEFF load under the PJRT path; all bf16 payloads
    # travel as int16 containers and are bitcast on-chip.
    nc = bacc.Bacc("TRN2", target_bir_lowering=False, debug=False,
                   num_devices=n_cores)
    gh_d = nc.dram_tensor("gh", [128, nchunks, F], i16,
                          kind="ExternalInput").ap()
    meta_d = nc.dram_tensor("meta", [128, nchunks, 4], i16,
                            kind="ExternalInput").ap()
    basesw_d = nc.dram_tensor("basesw", [128, NB, 2, 2, 128], i16,
                              kind="ExternalInput").ap()
    bias_d = nc.dram_tensor("bias", [128, 2], f32, kind="ExternalInput").ap()
    out_d = nc.dram_tensor("out", [2, 128, dcore], i16,
                           kind="ExternalOutput").ap()

    # chunk index -> (slot, is_first_chunk_of_slot, is_last_chunk_of_slot)
    chunk_slot = []
    for s, cw in enumerate(slot_cws):
        for c in range(cw):
            chunk_slot.append((s, c == 0, c == cw - 1))

    with tile.TileContext(nc) as tc:
        with (
            tc.tile_pool(name="const", bufs=1) as cpool,
            tc.tile_pool(name="gh", bufs=3) as gpool,
            tc.tile_pool(name="w1h", bufs=2) as wpool,
            tc.tile_pool(name="oh", bufs=2) as ohpool,
            tc.tile_pool(name="stg", bufs=2) as apool,
            tc.tile_pool(name="ost", bufs=4) as opool,
            tc.tile_pool(name="ps1", bufs=2, space="PSUM") as ps1,
            tc.tile_pool(name="ps2", bufs=2, space="PSUM") as ps2,
        ):
            # ---- prologue: constants ----
            # slab 0's first metadata slice + G data go first so the PE can
            # start as early as possible; the rest of the constants follow.
            t0 = min(nchunks, TSLAB)
            p0 = min(8, t0)
            meta_sb = cpool.tile([128, nchunks, 4], i16)
            nc.sync.dma_start(meta_sb[:, 0:p0, :], meta_d[:, 0:p0, :])
            meta_bf = meta_sb[:].bitcast(bf16)
            iota_sb = cpool.tile([128, W], bf16)
            nc.gpsimd.iota(iota_sb[:], pattern=[[1, W]], base=0,
                           channel_multiplier=0,
                           allow_small_or_imprecise_dtypes=True)

            slab_tiles = {}

            def build_w1h(w1h, oh, lo, hi, base):
                """DVE build of w1h[:, lo-base:hi-base] from meta[lo:hi]."""
                t = hi - lo
                a, b = lo - base, hi - base
                dstl = meta_bf[:, lo:hi, 0:1].broadcast_to([128, t, W])
                iota_b = iota_sb[:].unsqueeze(1).broadcast_to([128, t, W])
                nc.vector.tensor_tensor(oh[:, a:b, :], iota_b, dstl,
                                        mybir.AluOpType.is_equal)
                oh_b = oh[:, a:b, :].unsqueeze(2).broadcast_to(
                    [128, t, NB, W])
                cv_b = meta_bf[:, lo:hi, 1:4].unsqueeze(3).broadcast_to(
                    [128, t, NB, W])
                nc.vector.tensor_tensor(w1h[:, a:b, :, :], oh_b, cv_b,
                                        mybir.AluOpType.mult)

            def ensure_slab(si):
                """Issue the G-slab DMA + on-chip w1h build for slab si."""
                if si in slab_tiles or si >= nslabs:
                    return
                lo = si * TSLAB
                hi = min(nchunks, lo + TSLAB)
                t = hi - lo
                G = gpool.tile([128, TSLAB, F], i16, tag="G", name="G")
                mid = (t + 1) // 2
                nc.sync.dma_start(G[:, 0:mid, :], gh_d[:, lo:lo + mid, :])
                nc.sync.dma_start(G[:, mid:t, :], gh_d[:, lo + mid:hi, :])
                oh = ohpool.tile([128, TSLAB, W], bf16, tag="oh", name="oh")
                w1h = wpool.tile([128, TSLAB, NB, W], bf16, tag="W", name="W")
                build_w1h(w1h, oh, lo, hi, lo)
                slab_tiles[si] = (G, w1h)

            def emit_stage2_mm(g):
                """Matmul group g's staged features against the bases."""
                stg = staging[g]
                for ohalf in range(2):
                    p2 = ps2.tile([128, GRP * W], f32, tag=f"p2o{ohalf}",
                                  name=f"p2o{ohalf}")
                    k = 0
                    for b in range(NB):
                        for h in range(2):
                            nc.tensor.matmul(
                                p2[:],
                                basesw_sb[:, b, h, ohalf, :].bitcast(bf16),
                                stg[:, h, b, :, :],
                                start=(k == 0), stop=(k == 2 * NB - 1))
                            k += 1
                    p2_of_group[(g, ohalf)] = p2
                del staging[g]

            def emit_relu_out(g):
                """ReLU+bias and output DMA for group g (deferred so the
                scalar queue's casts never sit behind a relu)."""
                for ohalf in range(2):
                    p2 = p2_of_group.pop((g, ohalf))
                    osb = opool.tile([128, GRP * W], bf16, tag=f"osb{ohalf}",
                                     name=f"osb{ohalf}")
                    nc.scalar.activation(
                        osb[:], p2[:], mybir.ActivationFunctionType.Relu,
                        bias=bias_sb[:, ohalf:ohalf + 1])
                    # dispatch from sync so the ACT queue stays cast-only
                    nc.sync.dma_start(
                        out_d[ohalf, :, g * GRP * W:(g + 1) * GRP * W],
                        osb[:].bitcast(i16))

            # ---- main pipeline over window pairs ----
            staging = {}
            p2_of_group = {}
            cglob = 0
            # slab 0 is special-cased at 8-chunk granularity: the first MMs
            # can start after only 8 chunks of G + metadata have landed.
            G0 = gpool.tile([128, TSLAB, F], i16, tag="G", name="G")
            oh0 = ohpool.tile([128, TSLAB, W], bf16, tag="oh", name="oh")
            w1h0 = wpool.tile([128, TSLAB, NB, W], bf16, tag="W", name="W")
            nc.sync.dma_start(G0[:, 0:p0, :], gh_d[:, 0:p0, :])
            build_w1h(w1h0, oh0, 0, p0, 0)
            if t0 > p0:
                nc.sync.dma_start(meta_sb[:, p0:t0, :], meta_d[:, p0:t0, :])
            for lo in range(p0, t0, 8):
                hi = min(t0, lo + 8)
                nc.sync.dma_start(G0[:, lo:hi, :], gh_d[:, lo:hi, :])
                build_w1h(w1h0, oh0, lo, hi, 0)
            slab_tiles[0] = (G0, w1h0)
            basesw_sb = cpool.tile([128, NB, 2, 2, 128], i16)
            nc.sync.dma_start(basesw_sb[:], basesw_d[:])
            bias_sb = cpool.tile([128, 2], f32)
            nc.sync.dma_start(bias_sb[:], bias_d[:])
            if nchunks > t0:
                nc.sync.dma_start(meta_sb[:, t0:, :], meta_d[:, t0:, :])
            ensure_slab(1)
            for j in range(npairs):
                p1 = [ps1.tile([128, PW, NB, W], f32, tag=f"p1h{h}",
                               name=f"p1h{h}") for h in range(2)]
                for w01 in range(PW):
                    s = PW * j + w01
                    for c in range(slot_cws[s]):
                        si, cloc = divmod(cglob, TSLAB)
                        ensure_slab(si)
                        ensure_slab(si + 1)
                        G, w1h = slab_tiles[si]
                        first = (c == 0)
                        last = (c == slot_cws[s] - 1)
                        for h in range(2):
                            nc.tensor.matmul(
                                p1[h][:, w01, :, :],
                                G[:, cloc, h * 128:(h + 1) * 128].bitcast(
                                    bf16),
                                w1h[:, cloc, :, :],
                                start=first, stop=last)
                        cglob += 1
                # defer stage2 of group (j-2)//2 to keep PE fed
                if j >= 2 and j % 2 == 0:
                    emit_stage2_mm((j - 2) // 2)
                if j >= 3 and j % 2 == 1:
                    emit_relu_out((j - 3) // 2)
                g = j // 2
                if g not in staging:
                    staging[g] = apool.tile([128, 2, NB, GRP, W], bf16,
                                            tag="stg", name="stg")
                for h in range(2):
                    a = PW * (j % 2)
                    dst_ap = staging[g][:, h, :, a:a + PW,
                                        :].transpose([0, 2, 1, 3])
                    # psum->staging casts ride the mostly-idle scalar engine
                    nc.scalar.copy(dst_ap, p1[h][:])
            for g in sorted(staging):
                emit_stage2_mm(g)
            for g in sorted(set(g for g, _ in p2_of_group)):
                emit_relu_out(g)

    nc.compile()
    return nc


def host_prep(h_bf, src, dst, rel, comp, n_nodes, n_cores):
    """Sort/deal/pad edges; pre-gather h and build the metadata stream."""
    dcore = n_nodes // n_cores
    nw = dcore // W
    ngw = n_cores * nw
    gw = (dst // W).astype(np.int64)
    order = np.argsort(gw, kind="stable")
    counts = np.bincount(gw, minlength=ngw)
    starts = np.concatenate([[0], np.cumsum(counts)])

    # deal windows to cores by descending count; slot capacity = group max
    ranked = np.argsort(-counts, kind="stable")
    slot_cws = [max(1, -(-int(counts[ranked[n_cores * i]]) // 128))
                for i in range(nw)]
    nchunks = sum(slot_cws)
    epad = nchunks * 128

    w_edge = comp[rel].astype(ml_dtypes.bfloat16)        # [E, NB]
    dstloc = (dst % W).astype(np.float32).astype(ml_dtypes.bfloat16)

    gh = np.zeros((n_cores, 128, nchunks, F), np.int16)
    meta = np.zeros((n_cores, 128, nchunks, 4), ml_dtypes.bfloat16)
    win_of_slot = np.zeros((n_cores, nw), np.int64)

    slot_base = np.concatenate([[0], np.cumsum(slot_cws)])[:-1]
    srcs_flat = np.zeros((n_cores, epad), np.int64)
    meta_flat = np.zeros((n_cores, epad, 4), ml_dtypes.bfloat16)
    valid = np.zeros((n_cores, epad), bool)
    for k in range(n_cores):
        for i in range(nw):
            wid = int(ranked[n_cores * i + k])
            win_of_slot[k, i] = wid
            es = order[starts[wid]:starts[wid + 1]]
            base = slot_base[i] * 128
            n = len(es)
            srcs_flat[k, base:base + n] = src[es]
            valid[k, base:base + n] = True
            meta_flat[k, base:base + n, 0] = dstloc[es]
            meta_flat[k, base:base + n, 1:4] = w_edge[es]
    for k in range(n_cores):
        g = h_bf[srcs_flat[k]]                           # [epad, F] int16
        g[~valid[k]] = 0
        gh[k] = g.reshape(nchunks, 128, F).transpose(1, 0, 2)
        meta[k] = meta_flat[k].reshape(nchunks, 128, 4).transpose(1, 0, 2)
    return gh, meta, tuple(slot_cws), win_of_slot


def rgcn_kernel(text, src, dst, rel, bases, comp, bias, n_cores=8,
                run_fn=None, nc_cache={}):
    """Full-input kernel: shard, run on 8 cores, reassemble output."""
    Bt, St, INF = text.shape
    n_nodes = Bt * St
    h = text.reshape(n_nodes, INF)

    src = np.asarray(src).astype(np.int64)
    dst = np.asarray(dst).astype(np.int64)
    rel = np.asarray(rel).astype(np.int64)
    bases_np = np.asarray(bases, np.float32)
    comp_np = np.asarray(comp, np.float32)
    bias_np = np.asarray(bias, np.float32)

    h_bf = np.asarray(h, np.float32).astype(ml_dtypes.bfloat16).view(np.int16)
    gh, meta, slot_cws, win_of_slot = host_prep(
        h_bf, src, dst, rel, comp_np, n_nodes, n_cores)
    key = (n_nodes, slot_cws, n_cores)
    if key not in nc_cache:
        nc_cache[key] = build_program(n_nodes, slot_cws, n_cores)
    nc = nc_cache[key]

    # bases[b, f, o] -> basesw[p, b, h, oh, q] with f = h*128+p, o = oh*128+q
    bw = bases_np.astype(ml_dtypes.bfloat16).view(np.int16)
    basesw = np.ascontiguousarray(
        bw.reshape(NB, 2, 128, 2, 128).transpose(2, 0, 1, 3, 4))
    bias_w = np.ascontiguousarray(
        bias_np.reshape(2, 128).T.astype(np.float32))

    in_maps = [
        dict(gh=gh[k], meta=meta[k].view(np.int16), basesw=basesw,
             bias=bias_w)
        for k in range(n_cores)
    ]
    from concourse.bass_utils import run_bass_kernel_spmd
    if run_fn is None:
        res = run_bass_kernel_spmd(nc, in_maps, list(range(n_cores)))
        outs = [res.results[k]["out"] for k in range(n_cores)]
    else:
        outs = run_fn(nc, in_maps)

    out = np.zeros((n_nodes, O), np.float32)
    nw = len(slot_cws)
    for k in range(n_cores):
        ok = outs[k].view(ml_dtypes.bfloat16).astype(np.float32)
        ok = ok.reshape(O, nw, W)                        # [o, slot, d]
        for i in range(nw):
            wid = win_of_slot[k][i]
            out[wid * W:(wid + 1) * W] = ok[:, i, :].T
    return out.reshape(Bt, St, O)


_NC_CACHE = {}


def kernel(text, src, dst, rel, bases, comp, bias):
    out = rgcn_kernel(
        np.asarray(text, np.float32),
        np.asarray(src), np.asarray(dst), np.asarray(rel),
        np.asarray(bases, np.float32), np.asarray(comp, np.float32),
        np.asarray(bias, np.float32),
        n_cores=8, nc_cache=_NC_CACHE)
    return np.ascontiguousarray(out, np.float32)


# revision 37
# speedup vs baseline: 1.2609x; 1.0661x over previous
"""Self-contained TRN2 Bass kernel for the RGCN message-passing problem.

kernel(**inputs) takes the FULL unsharded inputs (text, src, dst, rel,
bases, comp, bias), shards edges by destination window across the 8
NeuronCores, runs the SPMD Bass program via run_bass_kernel_spmd, and
returns the full [64, 512, 256] float32 output.

Design (v2):
  - Edges are grouped by destination window (W=64 dst rows) and dealt
    to cores by descending window edge-count.  Since the edge indices
    are known on the host, the h[src] gather is done host-side: the
    device streams a pre-gathered [128, nchunks, 256] bf16 tensor with
    large sequential HWDGE DMAs (no SWDGE descriptor generation).
  - The per-edge scatter weights (comp[rel] placed at column
    b*W + dst%W) are built ON-CHIP from an 8-byte/edge metadata stream
    with two DVE ops per slab: onehot = is_equal(iota, dstloc) and
    w1h = onehot * compvals (broadcast APs).
  - Stage 1: per 128-edge chunk, 2 matmuls (G feature-half stationary,
    w1h moving) accumulate A[f, (w2,b,d)] for a PAIR of windows in one
    PSUM tile.
  - Stage 2: flipped — bases halves are the stationary operand (full
    128-wide PE), aggregated features stream as 256 dst columns per
    group of 4 windows.  Bias+ReLU fused in the scalar-engine
    activation; output written as [2, 128, dcore] (feature-major) and
    transposed back on the host.
"""

import numpy as np
import ml_dtypes

import concourse.bass as bass
import concourse.tile as tile
from concourse import bacc, mybir

F = 256       # in features
O = 256       # out features
NB = 3        # bases
W = 32        # dst rows per window
TSLAB = 32    # chunks per DMA slab (32 * 128 * 512B = 2 MiB)
GRP = 256 // W            # windows per stage-2 group (GRP*W = 256 dst cols)
PW = 384 // (NB * W)      # windows per stage-1 PSUM tile (384 f32 cols)


def build_program(n_nodes, slot_cws, n_cores=8):
    slot_cws = list(slot_cws)
    nw = len(slot_cws)
    assert nw % GRP == 0
    nchunks = sum(slot_cws)
    dcore = nw * W
    nslabs = -(-nchunks // TSLAB)
    npairs = nw // PW

    bf16 = mybir.dt.bfloat16
    f32 = mybir.dt.float32
    i16 = mybir.dt.int16

    # bf16 DRAM I/O breaks NEFF load under the PJRT path; all bf16 payloads
    # travel as int16 containers and are bitcast on-chip.
    nc = bacc.Bacc("TRN2", target_bir_lowering=False, debug=False,
                   num_devices=n_cores)
    gh_d = nc.dram_tensor("gh", [128, nchunks, F], i16,
                          kind="ExternalInput").ap()
    meta_d = nc.dram_tensor("meta", [128, nchunks, 4], i16,
                            kind="ExternalInput").ap()
    basesw_d = nc.dram_tensor("basesw", [128, NB, 2, 2, 128], i16,
                              kind="ExternalInput").ap()
    bias_d = nc.dram_tensor("bias", [128, 2], f32, kind="ExternalInput").ap()
    out_d = nc.dram_tensor("out", [2, 128, dcore], i16,
                           kind="ExternalOutput").ap()

    # chunk index -> (slot, is_first_chunk_of_slot, is_last_chunk_of_slot)
    chunk_slot = []
    for s, cw in enumerate(slot_cws):
        for c in range(cw):
            chunk_slot.append((s, c == 0, c == cw - 1))

    with tile.TileContext(nc) as tc:
        with (
            tc.tile_pool(name="const", bufs=1) as cpool,
            tc.tile_pool(name="gh", bufs=4) as gpool,
            tc.tile_pool(name="w1h", bufs=2) as wpool,
            tc.tile_pool(name="oh", bufs=2) as ohpool,
            tc.tile_pool(name="stg", bufs=2) as apool,
            tc.tile_pool(name="ost", bufs=4) as opool,
            tc.tile_pool(name="ps1", bufs=2, space="PSUM") as ps1,
            tc.tile_pool(name="ps2", bufs=2, space="PSUM") as ps2,
        ):
            # ---- prologue: constants ----
            # slab 0's first metadata slice + G data go first so the PE can
            # start as early as possible; the rest of the constants follow.
            t0 = min(nchunks, TSLAB)
            p0 = min(8, t0)
            meta_sb = cpool.tile([128, nchunks, 4], i16)
            nc.sync.dma_start(meta_sb[:, 0:p0, :], meta_d[:, 0:p0, :])
            meta_bf = meta_sb[:].bitcast(bf16)
            iota_sb = cpool.tile([128, W], bf16)
            nc.gpsimd.iota(iota_sb[:], pattern=[[1, W]], base=0,
                           channel_multiplier=0,
                           allow_small_or_imprecise_dtypes=True)

            slab_tiles = {}

            def build_w1h(w1h, oh, lo, hi, base):
                """DVE build of w1h[:, lo-base:hi-base] from meta[lo:hi]."""
                t = hi - lo
                a, b = lo - base, hi - base
                dstl = meta_bf[:, lo:hi, 0:1].broadcast_to([128, t, W])
                iota_b = iota_sb[:].unsqueeze(1).broadcast_to([128, t, W])
                nc.vector.tensor_tensor(oh[:, a:b, :], iota_b, dstl,
                                        mybir.AluOpType.is_equal)
                oh_b = oh[:, a:b, :].unsqueeze(2).broadcast_to(
                    [128, t, NB, W])
                cv_b = meta_bf[:, lo:hi, 1:4].unsqueeze(3).broadcast_to(
                    [128, t, NB, W])
                nc.vector.tensor_tensor(w1h[:, a:b, :, :], oh_b, cv_b,
                                        mybir.AluOpType.mult)

            def ensure_slab(si):
                """Issue the G-slab DMA + on-chip w1h build for slab si."""
                if si in slab_tiles or si >= nslabs:
                    return
                lo = si * TSLAB
                hi = min(nchunks, lo + TSLAB)
                t = hi - lo
                G = gpool.tile([128, TSLAB, F], i16, tag="G", name="G")
                mid = (t + 1) // 2
                nc.sync.dma_start(G[:, 0:mid, :], gh_d[:, lo:lo + mid, :])
                nc.sync.dma_start(G[:, mid:t, :], gh_d[:, lo + mid:hi, :])
                oh = ohpool.tile([128, TSLAB, W], bf16, tag="oh", name="oh")
                w1h = wpool.tile([128, TSLAB, NB, W], bf16, tag="W", name="W")
                build_w1h(w1h, oh, lo, hi, lo)
                slab_tiles[si] = (G, w1h)

            def emit_stage2_mm(g):
                """Matmul group g's staged features against the bases."""
                stg = staging[g]
                for ohalf in range(2):
                    p2 = ps2.tile([128, GRP * W], f32, tag=f"p2o{ohalf}",
                                  name=f"p2o{ohalf}")
                    k = 0
                    for b in range(NB):
                        for h in range(2):
                            nc.tensor.matmul(
                                p2[:],
                                basesw_sb[:, b, h, ohalf, :].bitcast(bf16),
                                stg[:, h, b, :, :],
                                start=(k == 0), stop=(k == 2 * NB - 1))
                            k += 1
                    p2_of_group[(g, ohalf)] = p2
                del staging[g]

            def emit_relu_out(g):
                """ReLU+bias and output DMA for group g (deferred so the
                scalar queue's casts never sit behind a relu)."""
                for ohalf in range(2):
                    p2 = p2_of_group.pop((g, ohalf))
                    osb = opool.tile([128, GRP * W], bf16, tag=f"osb{ohalf}",
                                     name=f"osb{ohalf}")
                    nc.scalar.activation(
                        osb[:], p2[:], mybir.ActivationFunctionType.Relu,
                        bias=bias_sb[:, ohalf:ohalf + 1])
                    # dispatch from sync so the ACT queue stays cast-only
                    nc.sync.dma_start(
                        out_d[ohalf, :, g * GRP * W:(g + 1) * GRP * W],
                        osb[:].bitcast(i16))

            # ---- main pipeline over window pairs ----
            staging = {}
            p2_of_group = {}
            cglob = 0
            # slab 0 is special-cased at 8-chunk granularity: the first MMs
            # can start after only 8 chunks of G + metadata have landed.
            G0 = gpool.tile([128, TSLAB, F], i16, tag="G", name="G")
            oh0 = ohpool.tile([128, TSLAB, W], bf16, tag="oh", name="oh")
            w1h0 = wpool.tile([128, TSLAB, NB, W], bf16, tag="W", name="W")
            nc.sync.dma_start(G0[:, 0:p0, :], gh_d[:, 0:p0, :])
            build_w1h(w1h0, oh0, 0, p0, 0)
            if t0 > p0:
                nc.sync.dma_start(meta_sb[:, p0:t0, :], meta_d[:, p0:t0, :])
            for lo in range(p0, t0, 8):
                hi = min(t0, lo + 8)
                nc.sync.dma_start(G0[:, lo:hi, :], gh_d[:, lo:hi, :])
                build_w1h(w1h0, oh0, lo, hi, 0)
            slab_tiles[0] = (G0, w1h0)
            basesw_sb = cpool.tile([128, NB, 2, 2, 128], i16)
            nc.sync.dma_start(basesw_sb[:], basesw_d[:])
            bias_sb = cpool.tile([128, 2], f32)
            nc.sync.dma_start(bias_sb[:], bias_d[:])
            if nchunks > t0:
                nc.sync.dma_start(meta_sb[:, t0:, :], meta_d[:, t0:, :])
            ensure_slab(1)
            for j in range(npairs):
                p1 = [ps1.tile([128, PW, NB, W], f32, tag=f"p1h{h}",
                               name=f"p1h{h}") for h in range(2)]
                for w01 in range(PW):
                    s = PW * j + w01
                    for c in range(slot_cws[s]):
                        si, cloc = divmod(cglob, TSLAB)
                        ensure_slab(si)
                        ensure_slab(si + 1)
                        ensure_slab(si + 2)
                        G, w1h = slab_tiles[si]
                        first = (c == 0)
                        last = (c == slot_cws[s] - 1)
                        for h in range(2):
                            nc.tensor.matmul(
                                p1[h][:, w01, :, :],
                                G[:, cloc, h * 128:(h + 1) * 128].bitcast(
                                    bf16),
                                w1h[:, cloc, :, :],
                                start=first, stop=last)
                        cglob += 1
                # defer stage2 so its casts have two quads of PE cover
                if j >= 3 and j % 2 == 1:
                    emit_stage2_mm((j - 3) // 2)
                if j >= 4 and j % 2 == 0:
                    emit_relu_out((j - 4) // 2)
                g = j // 2
                if g not in staging:
                    staging[g] = apool.tile([128, 2, NB, GRP, W], bf16,
                                            tag="stg", name="stg")
                for h in range(2):
                    a = PW * (j % 2)
                    dst_ap = staging[g][:, h, :, a:a + PW,
                                        :].transpose([0, 2, 1, 3])
                    # psum->staging casts ride the mostly-idle scalar engine
                    nc.scalar.copy(dst_ap, p1[h][:])
            for g in sorted(staging):
                emit_stage2_mm(g)
            for g in sorted(set(g for g, _ in p2_of_group)):
                emit_relu_out(g)

    nc.compile()
    return nc


def host_prep(h_bf, src, dst, rel, comp, n_nodes, n_cores):
    """Sort/deal/pad edges; pre-gather h and build the metadata stream."""
    dcore = n_nodes // n_cores
    nw = dcore // W
    ngw = n_cores * nw
    gw = (dst // W).astype(np.int64)
    order = np.argsort(gw, kind="stable")
    counts = np.bincount(gw, minlength=ngw)
    starts = np.concatenate([[0], np.cumsum(counts)])

    # deal windows to cores by descending count; slot capacity = group max
    ranked = np.argsort(-counts, kind="stable")
    slot_cws = [max(1, -(-int(counts[ranked[n_cores * i]]) // 128))
                for i in range(nw)]
    nchunks = sum(slot_cws)
    epad = nchunks * 128

    w_edge = comp[rel].astype(ml_dtypes.bfloat16)        # [E, NB]
    dstloc = (dst % W).astype(np.float32).astype(ml_dtypes.bfloat16)

    gh = np.zeros((n_cores, 128, nchunks, F), np.int16)
    meta = np.zeros((n_cores, 128, nchunks, 4), ml_dtypes.bfloat16)
    win_of_slot = np.zeros((n_cores, nw), np.int64)

    slot_base = np.concatenate([[0], np.cumsum(slot_cws)])[:-1]
    srcs_flat = np.zeros((n_cores, epad), np.int64)
    meta_flat = np.zeros((n_cores, epad, 4), ml_dtypes.bfloat16)
    valid = np.zeros((n_cores, epad), bool)
    for k in range(n_cores):
        for i in range(nw):
            wid = int(ranked[n_cores * i + k])
            win_of_slot[k, i] = wid
            es = order[starts[wid]:starts[wid + 1]]
            base = slot_base[i] * 128
            n = len(es)
            srcs_flat[k, base:base + n] = src[es]
            valid[k, base:base + n] = True
            meta_flat[k, base:base + n, 0] = dstloc[es]
            meta_flat[k, base:base + n, 1:4] = w_edge[es]
    for k in range(n_cores):
        g = h_bf[srcs_flat[k]]                           # [epad, F] int16
        g[~valid[k]] = 0
        gh[k] = g.reshape(nchunks, 128, F).transpose(1, 0, 2)
        meta[k] = meta_flat[k].reshape(nchunks, 128, 4).transpose(1, 0, 2)
    return gh, meta, tuple(slot_cws), win_of_slot


def rgcn_kernel(text, src, dst, rel, bases, comp, bias, n_cores=8,
                run_fn=None, nc_cache={}):
    """Full-input kernel: shard, run on 8 cores, reassemble output."""
    Bt, St, INF = text.shape
    n_nodes = Bt * St
    h = text.reshape(n_nodes, INF)

    src = np.asarray(src).astype(np.int64)
    dst = np.asarray(dst).astype(np.int64)
    rel = np.asarray(rel).astype(np.int64)
    bases_np = np.asarray(bases, np.float32)
    comp_np = np.asarray(comp, np.float32)
    bias_np = np.asarray(bias, np.float32)

    h_bf = np.asarray(h, np.float32).astype(ml_dtypes.bfloat16).view(np.int16)
    gh, meta, slot_cws, win_of_slot = host_prep(
        h_bf, src, dst, rel, comp_np, n_nodes, n_cores)
    key = (n_nodes, slot_cws, n_cores)
    if key not in nc_cache:
        nc_cache[key] = build_program(n_nodes, slot_cws, n_cores)
    nc = nc_cache[key]

    # bases[b, f, o] -> basesw[p, b, h, oh, q] with f = h*128+p, o = oh*128+q
    bw = bases_np.astype(ml_dtypes.bfloat16).view(np.int16)
    basesw = np.ascontiguousarray(
        bw.reshape(NB, 2, 128, 2, 128).transpose(2, 0, 1, 3, 4))
    bias_w = np.ascontiguousarray(
        bias_np.reshape(2, 128).T.astype(np.float32))

    in_maps = [
        dict(gh=gh[k], meta=meta[k].view(np.int16), basesw=basesw,
             bias=bias_w)
        for k in range(n_cores)
    ]
    from concourse.bass_utils import run_bass_kernel_spmd
    if run_fn is None:
        res = run_bass_kernel_spmd(nc, in_maps, list(range(n_cores)))
        outs = [res.results[k]["out"] for k in range(n_cores)]
    else:
        outs = run_fn(nc, in_maps)

    out = np.zeros((n_nodes, O), np.float32)
    nw = len(slot_cws)
    for k in range(n_cores):
        ok = outs[k].view(ml_dtypes.bfloat16).astype(np.float32)
        ok = ok.reshape(O, nw, W)                        # [o, slot, d]
        for i in range(nw):
            wid = win_of_slot[k][i]
            out[wid * W:(wid + 1) * W] = ok[:, i, :].T
    return out.reshape(Bt, St, O)


_NC_CACHE = {}


def kernel(text, src, dst, rel, bases, comp, bias):
    out = rgcn_kernel(
        np.asarray(text, np.float32),
        np.asarray(src), np.asarray(dst), np.asarray(rel),
        np.asarray(bases, np.float32), np.asarray(comp, np.float32),
        np.asarray(bias, np.float32),
        n_cores=8, nc_cache=_NC_CACHE)
    return np.ascontiguousarray(out, np.float32)


# revision 40
# speedup vs baseline: 1.2654x; 1.0036x over previous
"""Self-contained TRN2 Bass kernel for the RGCN message-passing problem.

kernel(**inputs) takes the FULL unsharded inputs (text, src, dst, rel,
bases, comp, bias), shards edges by destination window across the 8
NeuronCores, runs the SPMD Bass program via run_bass_kernel_spmd, and
returns the full [64, 512, 256] float32 output.

Design (v2):
  - Edges are grouped by destination window (W=64 dst rows) and dealt
    to cores by descending window edge-count.  Since the edge indices
    are known on the host, the h[src] gather is done host-side: the
    device streams a pre-gathered [128, nchunks, 256] bf16 tensor with
    large sequential HWDGE DMAs (no SWDGE descriptor generation).
  - The per-edge scatter weights (comp[rel] placed at column
    b*W + dst%W) are built ON-CHIP from an 8-byte/edge metadata stream
    with two DVE ops per slab: onehot = is_equal(iota, dstloc) and
    w1h = onehot * compvals (broadcast APs).
  - Stage 1: per 128-edge chunk, 2 matmuls (G feature-half stationary,
    w1h moving) accumulate A[f, (w2,b,d)] for a PAIR of windows in one
    PSUM tile.
  - Stage 2: flipped — bases halves are the stationary operand (full
    128-wide PE), aggregated features stream as 256 dst columns per
    group of 4 windows.  Bias+ReLU fused in the scalar-engine
    activation; output written as [2, 128, dcore] (feature-major) and
    transposed back on the host.
"""

import numpy as np
import ml_dtypes

import concourse.bass as bass
import concourse.tile as tile
from concourse import bacc, mybir

F = 256       # in features
O = 256       # out features
NB = 3        # bases
W = 32        # dst rows per window
TSLAB = 32    # chunks per DMA slab (32 * 128 * 512B = 2 MiB)
GRP = 256 // W            # windows per stage-2 group (GRP*W = 256 dst cols)
PW = 384 // (NB * W)      # windows per stage-1 PSUM tile (384 f32 cols)


def build_program(n_nodes, slot_cws, n_cores=8):
    slot_cws = list(slot_cws)
    nw = len(slot_cws)
    assert nw % GRP == 0
    nchunks = sum(slot_cws)
    dcore = nw * W
    nslabs = -(-nchunks // TSLAB)
    npairs = nw // PW

    bf16 = mybir.dt.bfloat16
    f32 = mybir.dt.float32
    i16 = mybir.dt.int16

    # bf16 DRAM I/O breaks NEFF load under the PJRT path; all bf16 payloads
    # travel as int16 containers and are bitcast on-chip.
    nc = bacc.Bacc("TRN2", target_bir_lowering=False, debug=False,
                   num_devices=n_cores)
    gh_d = nc.dram_tensor("gh", [128, nchunks, F], i16,
                          kind="ExternalInput").ap()
    meta_d = nc.dram_tensor("meta", [128, nchunks, 4], i16,
                            kind="ExternalInput").ap()
    basesw_d = nc.dram_tensor("basesw", [128, NB, 2, 2, 128], i16,
                              kind="ExternalInput").ap()
    bias_d = nc.dram_tensor("bias", [128, 2], f32, kind="ExternalInput").ap()
    out_d = nc.dram_tensor("out", [2, 128, dcore], i16,
                           kind="ExternalOutput").ap()

    # chunk index -> (slot, is_first_chunk_of_slot, is_last_chunk_of_slot)
    chunk_slot = []
    for s, cw in enumerate(slot_cws):
        for c in range(cw):
            chunk_slot.append((s, c == 0, c == cw - 1))

    with tile.TileContext(nc) as tc:
        with (
            tc.tile_pool(name="const", bufs=1) as cpool,
            tc.tile_pool(name="gh", bufs=4) as gpool,
            tc.tile_pool(name="w1h", bufs=2) as wpool,
            tc.tile_pool(name="oh", bufs=2) as ohpool,
            tc.tile_pool(name="stg", bufs=2) as apool,
            tc.tile_pool(name="ost", bufs=4) as opool,
            tc.tile_pool(name="ps1", bufs=2, space="PSUM") as ps1,
            tc.tile_pool(name="ps2", bufs=2, space="PSUM") as ps2,
        ):
            # ---- prologue: constants ----
            # slab 0's first metadata slice + G data go first so the PE can
            # start as early as possible; the rest of the constants follow.
            t0 = min(nchunks, TSLAB)
            p0 = min(8, t0)
            meta_sb = cpool.tile([128, nchunks, 4], i16)
            # scalar ring: lands in parallel with G slab 0 on the SP ring
            nc.scalar.dma_start(meta_sb[:, 0:p0, :], meta_d[:, 0:p0, :])
            meta_bf = meta_sb[:].bitcast(bf16)
            iota_sb = cpool.tile([128, W], bf16)
            nc.gpsimd.iota(iota_sb[:], pattern=[[1, W]], base=0,
                           channel_multiplier=0,
                           allow_small_or_imprecise_dtypes=True)

            slab_tiles = {}

            def build_w1h(w1h, oh, lo, hi, base):
                """DVE build of w1h[:, lo-base:hi-base] from meta[lo:hi]."""
                t = hi - lo
                a, b = lo - base, hi - base
                dstl = meta_bf[:, lo:hi, 0:1].broadcast_to([128, t, W])
                iota_b = iota_sb[:].unsqueeze(1).broadcast_to([128, t, W])
                nc.vector.tensor_tensor(oh[:, a:b, :], iota_b, dstl,
                                        mybir.AluOpType.is_equal)
                oh_b = oh[:, a:b, :].unsqueeze(2).broadcast_to(
                    [128, t, NB, W])
                cv_b = meta_bf[:, lo:hi, 1:4].unsqueeze(3).broadcast_to(
                    [128, t, NB, W])
                nc.vector.tensor_tensor(w1h[:, a:b, :, :], oh_b, cv_b,
                                        mybir.AluOpType.mult)

            def ensure_slab(si):
                """Issue the G-slab DMA + on-chip w1h build for slab si."""
                if si in slab_tiles or si >= nslabs:
                    return
                lo = si * TSLAB
                hi = min(nchunks, lo + TSLAB)
                t = hi - lo
                G = gpool.tile([128, TSLAB, F], i16, tag="G", name="G")
                mid = (t + 1) // 2
                oh = ohpool.tile([128, TSLAB, W], bf16, tag="oh", name="oh")
                w1h = wpool.tile([128, TSLAB, NB, W], bf16, tag="W", name="W")
                nc.sync.dma_start(G[:, 0:mid, :], gh_d[:, lo:lo + mid, :])
                build_w1h(w1h, oh, lo, lo + mid, lo)
                nc.sync.dma_start(G[:, mid:t, :], gh_d[:, lo + mid:hi, :])
                build_w1h(w1h, oh, lo + mid, hi, lo)
                slab_tiles[si] = (G, w1h)

            def emit_stage2_mm(g):
                """Matmul group g's staged features against the bases."""
                stg = staging[g]
                for ohalf in range(2):
                    p2 = ps2.tile([128, GRP * W], f32, tag=f"p2o{ohalf}",
                                  name=f"p2o{ohalf}")
                    k = 0
                    for b in range(NB):
                        for h in range(2):
                            nc.tensor.matmul(
                                p2[:],
                                basesw_sb[:, b, h, ohalf, :].bitcast(bf16),
                                stg[:, h, b, :, :],
                                start=(k == 0), stop=(k == 2 * NB - 1))
                            k += 1
                    p2_of_group[(g, ohalf)] = p2
                del staging[g]

            def emit_relu_out(g):
                """ReLU+bias and output DMA for group g (deferred so the
                scalar queue's casts never sit behind a relu)."""
                for ohalf in range(2):
                    p2 = p2_of_group.pop((g, ohalf))
                    osb = opool.tile([128, GRP * W], bf16, tag=f"osb{ohalf}",
                                     name=f"osb{ohalf}")
                    nc.scalar.activation(
                        osb[:], p2[:], mybir.ActivationFunctionType.Relu,
                        bias=bias_sb[:, ohalf:ohalf + 1])
                    # dispatch from sync so the ACT queue stays cast-only
                    nc.sync.dma_start(
                        out_d[ohalf, :, g * GRP * W:(g + 1) * GRP * W],
                        osb[:].bitcast(i16))

            # ---- main pipeline over window pairs ----
            staging = {}
            p2_of_group = {}
            cglob = 0
            # slab 0 is special-cased at 8-chunk granularity: the first MMs
            # can start after only 8 chunks of G + metadata have landed.
            G0 = gpool.tile([128, TSLAB, F], i16, tag="G", name="G")
            oh0 = ohpool.tile([128, TSLAB, W], bf16, tag="oh", name="oh")
            w1h0 = wpool.tile([128, TSLAB, NB, W], bf16, tag="W", name="W")
            nc.sync.dma_start(G0[:, 0:p0, :], gh_d[:, 0:p0, :])
            build_w1h(w1h0, oh0, 0, p0, 0)
            if t0 > p0:
                nc.sync.dma_start(meta_sb[:, p0:t0, :], meta_d[:, p0:t0, :])
            for lo in range(p0, t0, 8):
                hi = min(t0, lo + 8)
                nc.sync.dma_start(G0[:, lo:hi, :], gh_d[:, lo:hi, :])
                build_w1h(w1h0, oh0, lo, hi, 0)
            slab_tiles[0] = (G0, w1h0)
            basesw_sb = cpool.tile([128, NB, 2, 2, 128], i16)
            nc.sync.dma_start(basesw_sb[:], basesw_d[:])
            bias_sb = cpool.tile([128, 2], f32)
            nc.sync.dma_start(bias_sb[:], bias_d[:])
            if nchunks > t0:
                nc.sync.dma_start(meta_sb[:, t0:, :], meta_d[:, t0:, :])
            ensure_slab(1)
            for j in range(npairs):
                p1 = [ps1.tile([128, PW, NB, W], f32, tag=f"p1h{h}",
                               name=f"p1h{h}") for h in range(2)]
                for w01 in range(PW):
                    s = PW * j + w01
                    for c in range(slot_cws[s]):
                        si, cloc = divmod(cglob, TSLAB)
                        ensure_slab(si)
                        ensure_slab(si + 1)
                        ensure_slab(si + 2)
                        G, w1h = slab_tiles[si]
                        first = (c == 0)
                        last = (c == slot_cws[s] - 1)
                        for h in range(2):
                            nc.tensor.matmul(
                                p1[h][:, w01, :, :],
                                G[:, cloc, h * 128:(h + 1) * 128].bitcast(
                                    bf16),
                                w1h[:, cloc, :, :],
                                start=first, stop=last)
                        cglob += 1
                # defer stage2 so its casts have two quads of PE cover
                if j >= 3 and j % 2 == 1:
                    emit_stage2_mm((j - 3) // 2)
                if j >= 4 and j % 2 == 0:
                    emit_relu_out((j - 4) // 2)
                g = j // 2
                if g not in staging:
                    staging[g] = apool.tile([128, 2, NB, GRP, W], bf16,
                                            tag="stg", name="stg")
                for h in range(2):
                    a = PW * (j % 2)
                    dst_ap = staging[g][:, h, :, a:a + PW,
                                        :].transpose([0, 2, 1, 3])
                    # psum->staging casts ride the mostly-idle scalar engine
                    nc.scalar.copy(dst_ap, p1[h][:])
            pending = sorted(set(g for g, _ in p2_of_group))
            for g in pending:
                emit_relu_out(g)
            for g in sorted(staging):
                emit_stage2_mm(g)
                emit_relu_out(g)

    nc.compile()
    return nc


def host_prep(h_bf, src, dst, rel, comp, n_nodes, n_cores):
    """Sort/deal/pad edges; pre-gather h and build the metadata stream."""
    dcore = n_nodes // n_cores
    nw = dcore // W
    ngw = n_cores * nw
    gw = (dst // W).astype(np.int64)
    order = np.argsort(gw, kind="stable")
    counts = np.bincount(gw, minlength=ngw)
    starts = np.concatenate([[0], np.cumsum(counts)])

    # deal windows to cores by descending count; slot capacity = group max
    ranked = np.argsort(-counts, kind="stable")
    slot_cws = [max(1, -(-int(counts[ranked[n_cores * i]]) // 128))
                for i in range(nw)]
    nchunks = sum(slot_cws)
    epad = nchunks * 128

    w_edge = comp[rel].astype(ml_dtypes.bfloat16)        # [E, NB]
    dstloc = (dst % W).astype(np.float32).astype(ml_dtypes.bfloat16)

    gh = np.zeros((n_cores, 128, nchunks, F), np.int16)
    meta = np.zeros((n_cores, 128, nchunks, 4), ml_dtypes.bfloat16)
    win_of_slot = np.zeros((n_cores, nw), np.int64)

    slot_base = np.concatenate([[0], np.cumsum(slot_cws)])[:-1]
    srcs_flat = np.zeros((n_cores, epad), np.int64)
    meta_flat = np.zeros((n_cores, epad, 4), ml_dtypes.bfloat16)
    valid = np.zeros((n_cores, epad), bool)
    for k in range(n_cores):
        for i in range(nw):
            wid = int(ranked[n_cores * i + k])
            win_of_slot[k, i] = wid
            es = order[starts[wid]:starts[wid + 1]]
            base = slot_base[i] * 128
            n = len(es)
            srcs_flat[k, base:base + n] = src[es]
            valid[k, base:base + n] = True
            meta_flat[k, base:base + n, 0] = dstloc[es]
            meta_flat[k, base:base + n, 1:4] = w_edge[es]
    for k in range(n_cores):
        g = h_bf[srcs_flat[k]]                           # [epad, F] int16
        g[~valid[k]] = 0
        gh[k] = g.reshape(nchunks, 128, F).transpose(1, 0, 2)
        meta[k] = meta_flat[k].reshape(nchunks, 128, 4).transpose(1, 0, 2)
    return gh, meta, tuple(slot_cws), win_of_slot


def rgcn_kernel(text, src, dst, rel, bases, comp, bias, n_cores=8,
                run_fn=None, nc_cache={}):
    """Full-input kernel: shard, run on 8 cores, reassemble output."""
    Bt, St, INF = text.shape
    n_nodes = Bt * St
    h = text.reshape(n_nodes, INF)

    src = np.asarray(src).astype(np.int64)
    dst = np.asarray(dst).astype(np.int64)
    rel = np.asarray(rel).astype(np.int64)
    bases_np = np.asarray(bases, np.float32)
    comp_np = np.asarray(comp, np.float32)
    bias_np = np.asarray(bias, np.float32)

    h_bf = np.asarray(h, np.float32).astype(ml_dtypes.bfloat16).view(np.int16)
    gh, meta, slot_cws, win_of_slot = host_prep(
        h_bf, src, dst, rel, comp_np, n_nodes, n_cores)
    key = (n_nodes, slot_cws, n_cores)
    if key not in nc_cache:
        nc_cache[key] = build_program(n_nodes, slot_cws, n_cores)
    nc = nc_cache[key]

    # bases[b, f, o] -> basesw[p, b, h, oh, q] with f = h*128+p, o = oh*128+q
    bw = bases_np.astype(ml_dtypes.bfloat16).view(np.int16)
    basesw = np.ascontiguousarray(
        bw.reshape(NB, 2, 128, 2, 128).transpose(2, 0, 1, 3, 4))
    bias_w = np.ascontiguousarray(
        bias_np.reshape(2, 128).T.astype(np.float32))

    in_maps = [
        dict(gh=gh[k], meta=meta[k].view(np.int16), basesw=basesw,
             bias=bias_w)
        for k in range(n_cores)
    ]
    from concourse.bass_utils import run_bass_kernel_spmd
    if run_fn is None:
        res = run_bass_kernel_spmd(nc, in_maps, list(range(n_cores)))
        outs = [res.results[k]["out"] for k in range(n_cores)]
    else:
        outs = run_fn(nc, in_maps)

    out = np.zeros((n_nodes, O), np.float32)
    nw = len(slot_cws)
    for k in range(n_cores):
        ok = outs[k].view(ml_dtypes.bfloat16).astype(np.float32)
        ok = ok.reshape(O, nw, W)                        # [o, slot, d]
        for i in range(nw):
            wid = win_of_slot[k][i]
            out[wid * W:(wid + 1) * W] = ok[:, i, :].T
    return out.reshape(Bt, St, O)


_NC_CACHE = {}


def kernel(text, src, dst, rel, bases, comp, bias):
    out = rgcn_kernel(
        np.asarray(text, np.float32),
        np.asarray(src), np.asarray(dst), np.asarray(rel),
        np.asarray(bases, np.float32), np.asarray(comp, np.float32),
        np.asarray(bias, np.float32),
        n_cores=8, nc_cache=_NC_CACHE)
    return np.ascontiguousarray(out, np.float32)
